# revision 25
# baseline (speedup 1.0000x reference)
"""Trainium2 Bass kernel for nn_KeywordsLoss.

Computes: KLDivLoss(batchmean) between target = softmax(scatter(alpha at
keyword positions)) and logp = log_softmax(mean_s(logits) with [:,0]=0).

Closed form (per batch row b, V=50257, alpha=0.9):
  K_b   = unique non-zero keyword ids (special ids remapped to 0, excluded)
  k_b   = |K_b|
  D_b   = (V - k_b) + k_b * e^a          (softmax denominator of the target)
  m     = mean_s logits[b],  m[0] = 0
  lse   = log sum_v exp(m)
  loss_b = [lse - log D_b] + a*k_b*e^a/D_b - sum(m)/D_b - (e^a-1)*sum_{K_b}(m)/D_b
  loss  = sum_b loss_b / B

Device computes per-row raw stats; everything else is tiny host math.

Sharding: data-parallel over B: 2 batch rows per core, 8 cores. Each core
returns per-partition stats [A, E, Wv] per row; host finishes in float64.

Device pipeline per batch row (the heavy 51.5MB/row is DMA-bound; the
seq-reduction runs on the Tensor engine so DVE/ACT stay nearly idle):
  - DMA tiles [128 seq x C vocab] (contiguous C*4-byte runs per partition),
    two 128-row seq halves per batch row.
  - For each 128-wide vocab chunk: matmul(psum[:, col], lhsT=chunk,
    rhs=ones[128,1]) -> psum partition p, column col holds
    sum_s logits[s, col*128+p]. Halves accumulate in two separate PSUM
    banks (single-shot matmuls; robust to bank-level has_written clears).
  - Epilogue: acc = psA + psB (SBUF), zero the 47-pad tail + vocab 0,
    row-sum -> A, exp(acc/S2) with accum -> E, dot with keyword multi-hot
    -> Wv. Stats [128, 3] per row are DMA'd out raw; host reduces the 128
    partitions and applies the closed form.
"""

import sys
from contextlib import ExitStack

import numpy as np

if "/opt/trn_rl_repo" not in sys.path:
    sys.path.insert(0, "/opt/trn_rl_repo")

import concourse.bass as bass
import concourse.bacc as bacc
import concourse.mybir as mybir
import concourse.tile as tile
from concourse.bass_utils import run_bass_kernel_spmd

# Problem constants (hardcoded per the harness contract).
V = 50257
B = 16
S2 = 256
NCORES = 8
BLOC = B // NCORES          # batch rows per core = 2
NH = S2 // 128              # seq halves per row = 2
NCH = (V + 127) // 128      # 128-wide vocab chunks = 393 (last has 81)
MLAST = V - (NCH - 1) * 128  # 81 valid lanes in the final chunk
NPAD = NCH * 128 - V        # 47 spurious zeros that reach the exp-sum
C = 1024                    # vocab columns per DMA tile (4KB runs)
NT = (V + C - 1) // C       # tiles per seq half = 25
ALPHA = 0.9
SPECIAL = (101, 102, 117, 120, 0)

F32 = mybir.dt.float32
BF16 = mybir.dt.bfloat16

XLEN = BLOC * S2 * V        # per-core flat input (no padding needed)


def build_program():
    nc = bacc.Bacc("TRN2", target_bir_lowering=False, debug=False)
    x = nc.declare_dram_parameter("x", [1, XLEN], F32, isOutput=False)
    wh = nc.declare_dram_parameter("wh", [BLOC, 128, NCH], F32, isOutput=False)
    out = nc.declare_dram_parameter("out", [128, 8], F32, isOutput=True)

    AF = mybir.ActivationFunctionType
    ALU = mybir.AluOpType
    AX = mybir.AxisListType

    with tile.TileContext(nc) as tc, ExitStack() as ctx:
        io = ctx.enter_context(tc.tile_pool(name="io", bufs=10))
        iob = ctx.enter_context(tc.tile_pool(name="iob", bufs=10))
        scr = ctx.enter_context(tc.tile_pool(name="scr", bufs=2))
        accp = ctx.enter_context(tc.tile_pool(name="accp", bufs=2))
        wtp = ctx.enter_context(tc.tile_pool(name="wtp", bufs=2))
        sml = ctx.enter_context(tc.tile_pool(name="sml", bufs=1))
        psp = ctx.enter_context(
            tc.tile_pool(name="ps", bufs=1, space=bass.MemorySpace.PSUM)
        )

        ones = sml.tile([128, 1], BF16, tag="ones")
        nc.vector.memset(ones[:], 1.0)
        statsout = sml.tile([128, 8], F32, tag="statsout")

        for b in range(BLOC):
            ph = []
            wt = wtp.tile([128, NCH], F32, tag=f"wt{b}")
            # Issued via the (otherwise idle) gpsimd SWDGE ring so it does
            # not delay the input-tile stream on the sync HWDGE ring.
            nc.gpsimd.dma_start(wt[:], wh[b])
            for h in range(NH):
                ps = psp.tile([128, NCH], F32, tag=f"ps{b}h{h}")
                ph.append(ps)
                # Pre-zero the last column: the final matmul only writes
                # lanes 0..MLAST-1 there; lanes MLAST..127 must read as 0.
                nc.vector.memset(ps[:, NCH - 1 : NCH], 0.0)
            for t in range(NT):
                v0 = t * C
                ct = min(C, V - v0)
                # One DMA covers both 128-row seq halves of this vocab
                # range (two 4KB runs per partition): halves the
                # instruction/semaphore count at the optimal packet size.
                # SBUF layout: cols 0:ct = seq half 0, C:C+ct = half 1.
                tl = io.tile([128, 2 * C], F32, tag="io")
                src = bass.AP(
                    x, b * S2 * V + v0, [[V, 128], [128 * V, 2], [1, ct]]
                )
                if ct == C:
                    nc.sync.dma_start(tl[:], src)
                else:
                    nc.sync.dma_start(tl[:, 0 : 2 * ct], src)
                # bf16 copy, alternating between the scalar and vector
                # engines (both otherwise idle): fp32 stationary operands
                # disable fast-weight-load and choke the PE at
                # ~430ns/chunk; bf16 weights stream 2x/cycle. PSUM still
                # accumulates fp32, so only the input rounding (2^-9
                # relative) is lost -- far inside the error budget.
                tb = iob.tile([128, 2 * C], BF16, tag="iob")
                nc.scalar.activation(tb[:, 0:ct], tl[:, 0:ct], AF.Copy)
                nc.vector.tensor_copy(tb[:, ct : 2 * ct], tl[:, ct : 2 * ct])
                for c in range(0, ct, 128):
                    m = min(128, ct - c)
                    col = (v0 + c) // 128
                    nc.tensor.matmul(
                        ph[0][0:m, col : col + 1],
                        tb[:, c : c + m],
                        ones[:, 0:1],
                        start=True,
                        stop=True,
                    )
                    nc.tensor.matmul(
                        ph[1][0:m, col : col + 1],
                        tb[:, ct + c : ct + c + m],
                        ones[:, 0:1],
                        start=True,
                        stop=True,
                    )
            # Epilogue: fold the two seq halves, clean pads, reduce stats.
            acc = accp.tile([128, NCH], F32, tag=f"acc{b}")
            nc.vector.tensor_copy(acc[:], ph[0][:])
            nc.vector.tensor_add(acc[:], acc[:], ph[1][:])
            # Vocab 0: reference forces m[0] = 0.
            nc.vector.memset(acc[0:1, 0:1], 0.0)
            nc.vector.tensor_reduce(
                statsout[:, 4 * b : 4 * b + 1], acc[:], axis=AX.X, op=ALU.add
            )
            et = scr.tile([128, NCH], F32, tag="et")
            # exp(m) with the mean folded into the activation scale; accum_out
            # gives the per-partition row sum in the same instruction. The
            # NPAD zeros contribute exp(0)=1 each; host subtracts them.
            nc.scalar.activation(
                et[:],
                acc[:],
                AF.Exp,
                scale=1.0 / S2,
                accum_out=statsout[:, 4 * b + 1 : 4 * b + 2],
            )
            st2 = scr.tile([128, NCH], F32, tag="st2")
            nc.vector.tensor_mul(st2[:], wt[:], acc[:])
            nc.vector.tensor_reduce(
                statsout[:, 4 * b + 2 : 4 * b + 3], st2[:], axis=AX.X, op=ALU.add
            )
        nc.sync.dma_start(out[:], statsout[:])
    nc.compile()
    return nc


_NC = None


def _get_program():
    global _NC
    if _NC is None:
        _NC = build_program()
    return _NC


def keyword_counts_and_hot(keywords):
    """Per-row unique non-special keyword count + multi-hot in device layout.

    Device layout: vocab id v -> (partition v % 128, column v // 128).
    """
    kw = np.asarray(keywords)
    ks = np.zeros(B, np.int64)
    whot = np.zeros((B, 128, NCH), np.float32)
    for bb in range(B):
        row = kw[bb].astype(np.int64)
        row = np.where(np.isin(row, SPECIAL), 0, row)
        uniq = np.unique(row)
        uniq = uniq[uniq != 0]
        ks[bb] = len(uniq)
        whot[bb, uniq % 128, uniq // 128] = 1.0
    return ks, whot


def make_in_maps(logits, keywords):
    logits = np.ascontiguousarray(np.asarray(logits, dtype=np.float32))
    ks, whot = keyword_counts_and_hot(keywords)
    in_maps = []
    for c in range(NCORES):
        sl = slice(c * BLOC, (c + 1) * BLOC)
        in_maps.append(
            {
                "x": logits[sl].reshape(1, XLEN),
                "wh": whot[sl],
            }
        )
    return ks, in_maps


def loss_from_stats(results, ks):
    """Host finish: fold per-partition stats, apply closed form in float64."""
    ea = float(np.exp(ALPHA))
    total = 0.0
    for ci, r in enumerate(results):
        s = np.asarray(r["out"], np.float64)
        for bb in range(BLOC):
            g = ci * BLOC + bb
            a_raw = s[:, 4 * bb + 0].sum()
            e_raw = s[:, 4 * bb + 1].sum() - NPAD
            w_raw = s[:, 4 * bb + 2].sum()
            k = float(ks[g])
            d = (V - k) + k * ea
            lse = np.log(e_raw)
            sum_m = a_raw / S2
            sum_kw = w_raw / S2
            total += (
                lse
                - np.log(d)
                + ALPHA * k * ea / d
                - sum_m / d
                - (ea - 1.0) * sum_kw / d
            )
    return np.float32(total / B)


def kernel(logits, keywords):
    ks, in_maps = make_in_maps(logits, keywords)
    nc = _get_program()
    res = run_bass_kernel_spmd(nc, in_maps, list(range(NCORES)))
    return loss_from_stats(res.results, ks)


# revision 26
# speedup vs baseline: 1.1389x; 1.1389x over previous
"""Trainium2 Bass kernel for nn_KeywordsLoss.

Computes: KLDivLoss(batchmean) between target = softmax(scatter(alpha at
keyword positions)) and logp = log_softmax(mean_s(logits) with [:,0]=0).

Closed form (per batch row b, V=50257, alpha=0.9):
  K_b   = unique non-zero keyword ids (special ids remapped to 0, excluded)
  k_b   = |K_b|
  D_b   = (V - k_b) + k_b * e^a          (softmax denominator of the target)
  m     = mean_s logits[b],  m[0] = 0
  lse   = log sum_v exp(m)
  loss_b = [lse - log D_b] + a*k_b*e^a/D_b - sum(m)/D_b - (e^a-1)*sum_{K_b}(m)/D_b
  loss  = sum_b loss_b / B

Device computes per-row raw stats; everything else is tiny host math.

Sharding: data-parallel over B: 2 batch rows per core, 8 cores. Each core
returns per-partition stats [A, E, Wv] per row; host finishes in float64.

Device pipeline per batch row (the heavy 51.5MB/row is DMA-bound; the
seq-reduction runs on the Tensor engine so DVE/ACT stay nearly idle):
  - DMA tiles [128 seq x C vocab] (contiguous C*4-byte runs per partition),
    two 128-row seq halves per batch row.
  - For each 128-wide vocab chunk: matmul(psum[:, col], lhsT=chunk,
    rhs=ones[128,1]) -> psum partition p, column col holds
    sum_s logits[s, col*128+p]. Halves accumulate in two separate PSUM
    banks (single-shot matmuls; robust to bank-level has_written clears).
  - Epilogue: acc = psA + psB (SBUF), zero the 47-pad tail + vocab 0,
    row-sum -> A, exp(acc/S2) with accum -> E, dot with keyword multi-hot
    -> Wv. Stats [128, 3] per row are DMA'd out raw; host reduces the 128
    partitions and applies the closed form.
"""

import sys
from contextlib import ExitStack

import numpy as np

if "/opt/trn_rl_repo" not in sys.path:
    sys.path.insert(0, "/opt/trn_rl_repo")

import concourse.bass as bass
import concourse.bacc as bacc
import concourse.mybir as mybir
import concourse.tile as tile
from concourse.bass_utils import run_bass_kernel_spmd

# Problem constants (hardcoded per the harness contract).
V = 50257
B = 16
S2 = 256
NCORES = 8
BLOC = B // NCORES          # batch rows per core = 2
NH = S2 // 128              # seq halves per row = 2
NCH = (V + 127) // 128      # 128-wide vocab chunks = 393 (last has 81)
MLAST = V - (NCH - 1) * 128  # 81 valid lanes in the final chunk
NPAD = NCH * 128 - V        # 47 spurious zeros that reach the exp-sum
C = 1024                    # vocab columns per DMA tile (4KB runs)
NT = (V + C - 1) // C       # tiles per seq half = 25
ALPHA = 0.9
SPECIAL = (101, 102, 117, 120, 0)

F32 = mybir.dt.float32
BF16 = mybir.dt.bfloat16

XLEN = BLOC * S2 * V        # per-core flat input (no padding needed)


def build_program():
    nc = bacc.Bacc("TRN2", target_bir_lowering=False, debug=False)
    x = nc.declare_dram_parameter("x", [1, XLEN], F32, isOutput=False)
    wh = nc.declare_dram_parameter("wh", [BLOC, 128, NCH], F32, isOutput=False)
    out = nc.declare_dram_parameter("out", [128, 8], F32, isOutput=True)

    AF = mybir.ActivationFunctionType
    ALU = mybir.AluOpType
    AX = mybir.AxisListType

    with tile.TileContext(nc) as tc, ExitStack() as ctx:
        io = ctx.enter_context(tc.tile_pool(name="io", bufs=10))
        iob = ctx.enter_context(tc.tile_pool(name="iob", bufs=10))
        scr = ctx.enter_context(tc.tile_pool(name="scr", bufs=2))
        accp = ctx.enter_context(tc.tile_pool(name="accp", bufs=2))
        wtp = ctx.enter_context(tc.tile_pool(name="wtp", bufs=2))
        sml = ctx.enter_context(tc.tile_pool(name="sml", bufs=1))
        psp = ctx.enter_context(
            tc.tile_pool(name="ps", bufs=1, space=bass.MemorySpace.PSUM)
        )

        ones = sml.tile([128, 1], BF16, tag="ones")
        nc.vector.memset(ones[:], 1.0)
        statsout = sml.tile([128, 8], F32, tag="statsout")

        for b in range(BLOC):
            ph = []
            wt = wtp.tile([128, NCH], F32, tag=f"wt{b}")
            # Issued via the (otherwise idle) gpsimd SWDGE ring so it does
            # not delay the input-tile stream on the sync HWDGE ring.
            nc.gpsimd.dma_start(wt[:], wh[b])
            for h in range(NH):
                ps = psp.tile([128, NCH], F32, tag=f"ps{b}h{h}")
                ph.append(ps)
                # Pre-zero the last column: the final matmul only writes
                # lanes 0..MLAST-1 there; lanes MLAST..127 must read as 0.
                nc.vector.memset(ps[:, NCH - 1 : NCH], 0.0)
                for t in range(NT):
                    v0 = t * C
                    ct = min(C, V - v0)
                    tl = io.tile([128, C], F32, tag="io")
                    src = bass.AP(
                        x, b * S2 * V + h * 128 * V + v0, [[V, 128], [1, ct]]
                    )
                    nc.sync.dma_start(tl[:, 0:ct], src)
                    # bf16 copy, alternating between the scalar and vector
                    # engines (both otherwise idle): fp32 stationary operands
                    # disable fast-weight-load and choke the PE at
                    # ~430ns/chunk; bf16 weights stream 2x/cycle. PSUM still
                    # accumulates fp32, so only the input rounding (2^-9
                    # relative) is lost -- far inside the error budget.
                    tb = iob.tile([128, C], BF16, tag="iob")
                    if t % 2 == 0:
                        nc.scalar.activation(tb[:, 0:ct], tl[:, 0:ct], AF.Copy)
                    else:
                        nc.vector.tensor_copy(tb[:, 0:ct], tl[:, 0:ct])
                    for c in range(0, ct, 128):
                        m = min(128, ct - c)
                        col = (v0 + c) // 128
                        nc.tensor.matmul(
                            ps[0:m, col : col + 1],
                            tb[:, c : c + m],
                            ones[:, 0:1],
                            start=True,
                            stop=True,
                        )
            # Epilogue: fold the two seq halves, clean pads, reduce stats.
            acc = accp.tile([128, NCH], F32, tag=f"acc{b}")
            nc.vector.tensor_copy(acc[:], ph[0][:])
            nc.vector.tensor_add(acc[:], acc[:], ph[1][:])
            # Vocab 0: reference forces m[0] = 0.
            nc.vector.memset(acc[0:1, 0:1], 0.0)
            nc.vector.tensor_reduce(
                statsout[:, 4 * b : 4 * b + 1], acc[:], axis=AX.X, op=ALU.add
            )
            et = scr.tile([128, NCH], F32, tag="et")
            # exp(m) with the mean folded into the activation scale; accum_out
            # gives the per-partition row sum in the same instruction. The
            # NPAD zeros contribute exp(0)=1 each; host subtracts them.
            nc.scalar.activation(
                et[:],
                acc[:],
                AF.Exp,
                scale=1.0 / S2,
                accum_out=statsout[:, 4 * b + 1 : 4 * b + 2],
            )
            st2 = scr.tile([128, NCH], F32, tag="st2")
            nc.vector.tensor_mul(st2[:], wt[:], acc[:])
            nc.vector.tensor_reduce(
                statsout[:, 4 * b + 2 : 4 * b + 3], st2[:], axis=AX.X, op=ALU.add
            )
        nc.sync.dma_start(out[:], statsout[:])
    nc.compile()
    return nc


_NC = None


def _get_program():
    global _NC
    if _NC is None:
        _NC = build_program()
    return _NC


def keyword_counts_and_hot(keywords):
    """Per-row unique non-special keyword count + multi-hot in device layout.

    Device layout: vocab id v -> (partition v % 128, column v // 128).
    """
    kw = np.asarray(keywords)
    ks = np.zeros(B, np.int64)
    whot = np.zeros((B, 128, NCH), np.float32)
    for bb in range(B):
        row = kw[bb].astype(np.int64)
        row = np.where(np.isin(row, SPECIAL), 0, row)
        uniq = np.unique(row)
        uniq = uniq[uniq != 0]
        ks[bb] = len(uniq)
        whot[bb, uniq % 128, uniq // 128] = 1.0
    return ks, whot


def make_in_maps(logits, keywords):
    logits = np.ascontiguousarray(np.asarray(logits, dtype=np.float32))
    ks, whot = keyword_counts_and_hot(keywords)
    in_maps = []
    for c in range(NCORES):
        sl = slice(c * BLOC, (c + 1) * BLOC)
        in_maps.append(
            {
                "x": logits[sl].reshape(1, XLEN),
                "wh": whot[sl],
            }
        )
    return ks, in_maps


def loss_from_stats(results, ks):
    """Host finish: fold per-partition stats, apply closed form in float64."""
    ea = float(np.exp(ALPHA))
    total = 0.0
    for ci, r in enumerate(results):
        s = np.asarray(r["out"], np.float64)
        for bb in range(BLOC):
            g = ci * BLOC + bb
            a_raw = s[:, 4 * bb + 0].sum()
            e_raw = s[:, 4 * bb + 1].sum() - NPAD
            w_raw = s[:, 4 * bb + 2].sum()
            k = float(ks[g])
            d = (V - k) + k * ea
            lse = np.log(e_raw)
            sum_m = a_raw / S2
            sum_kw = w_raw / S2
            total += (
                lse
                - np.log(d)
                + ALPHA * k * ea / d
                - sum_m / d
                - (ea - 1.0) * sum_kw / d
            )
    return np.float32(total / B)


def kernel(logits, keywords):
    ks, in_maps = make_in_maps(logits, keywords)
    nc = _get_program()
    res = run_bass_kernel_spmd(nc, in_maps, list(range(NCORES)))
    return loss_from_stats(res.results, ks)


# revision 27
# speedup vs baseline: 1.1394x; 1.0004x over previous
"""Trainium2 Bass kernel for nn_KeywordsLoss.

Computes: KLDivLoss(batchmean) between target = softmax(scatter(alpha at
keyword positions)) and logp = log_softmax(mean_s(logits) with [:,0]=0).

Closed form (per batch row b, V=50257, alpha=0.9):
  K_b   = unique non-zero keyword ids (special ids remapped to 0, excluded)
  k_b   = |K_b|
  D_b   = (V - k_b) + k_b * e^a          (softmax denominator of the target)
  m     = mean_s logits[b],  m[0] = 0
  lse   = log sum_v exp(m)
  loss_b = [lse - log D_b] + a*k_b*e^a/D_b - sum(m)/D_b - (e^a-1)*sum_{K_b}(m)/D_b
  loss  = sum_b loss_b / B

Device computes per-row raw stats; everything else is tiny host math.

Sharding: data-parallel over B: 2 batch rows per core, 8 cores. Each core
returns per-partition stats [A, E, Wv] per row; host finishes in float64.

Device pipeline per batch row (the heavy 51.5MB/row is DMA-bound; the
seq-reduction runs on the Tensor engine so DVE/ACT stay nearly idle):
  - DMA tiles [128 seq x C vocab] (contiguous C*4-byte runs per partition),
    two 128-row seq halves per batch row.
  - For each 128-wide vocab chunk: matmul(psum[:, col], lhsT=chunk,
    rhs=ones[128,1]) -> psum partition p, column col holds
    sum_s logits[s, col*128+p]. Halves accumulate in two separate PSUM
    banks (single-shot matmuls; robust to bank-level has_written clears).
  - Epilogue: acc = psA + psB (SBUF), zero the 47-pad tail + vocab 0,
    row-sum -> A, exp(acc/S2) with accum -> E, dot with keyword multi-hot
    -> Wv. Stats [128, 3] per row are DMA'd out raw; host reduces the 128
    partitions and applies the closed form.
"""

import sys
from contextlib import ExitStack

import numpy as np

if "/opt/trn_rl_repo" not in sys.path:
    sys.path.insert(0, "/opt/trn_rl_repo")

import concourse.bass as bass
import concourse.bacc as bacc
import concourse.mybir as mybir
import concourse.tile as tile
from concourse.bass_utils import run_bass_kernel_spmd

# Problem constants (hardcoded per the harness contract).
V = 50257
B = 16
S2 = 256
NCORES = 8
BLOC = B // NCORES          # batch rows per core = 2
NH = S2 // 128              # seq halves per row = 2
NCH = (V + 127) // 128      # 128-wide vocab chunks = 393 (last has 81)
MLAST = V - (NCH - 1) * 128  # 81 valid lanes in the final chunk
NPAD = NCH * 128 - V        # 47 spurious zeros that reach the exp-sum
C = 1024                    # vocab columns per DMA tile (4KB runs)
NT = (V + C - 1) // C       # tiles per seq half = 25
ALPHA = 0.9
SPECIAL = (101, 102, 117, 120, 0)

F32 = mybir.dt.float32
BF16 = mybir.dt.bfloat16

XLEN = BLOC * S2 * V        # per-core flat input (no padding needed)


def build_program():
    nc = bacc.Bacc("TRN2", target_bir_lowering=False, debug=False)
    x = nc.declare_dram_parameter("x", [1, XLEN], F32, isOutput=False)
    wh = nc.declare_dram_parameter("wh", [BLOC, 128, NCH], F32, isOutput=False)
    out = nc.declare_dram_parameter("out", [128, 8], F32, isOutput=True)

    AF = mybir.ActivationFunctionType
    ALU = mybir.AluOpType
    AX = mybir.AxisListType

    with tile.TileContext(nc) as tc, ExitStack() as ctx:
        io = ctx.enter_context(tc.tile_pool(name="io", bufs=10))
        iob = ctx.enter_context(tc.tile_pool(name="iob", bufs=10))
        scr = ctx.enter_context(tc.tile_pool(name="scr", bufs=2))
        accp = ctx.enter_context(tc.tile_pool(name="accp", bufs=2))
        wtp = ctx.enter_context(tc.tile_pool(name="wtp", bufs=2))
        sml = ctx.enter_context(tc.tile_pool(name="sml", bufs=1))
        psp = ctx.enter_context(
            tc.tile_pool(name="ps", bufs=1, space=bass.MemorySpace.PSUM)
        )

        ones = sml.tile([128, 1], BF16, tag="ones")
        nc.vector.memset(ones[:], 1.0)
        statsout = sml.tile([128, 8], F32, tag="statsout")

        for b in range(BLOC):
            ph = []
            wt = wtp.tile([128, NCH], F32, tag=f"wt{b}")
            # Issued via the (otherwise idle) gpsimd SWDGE ring so it does
            # not delay the input-tile stream on the sync HWDGE ring.
            nc.gpsimd.dma_start(wt[:], wh[b])
            for h in range(NH):
                ps = psp.tile([128, NCH], F32, tag=f"ps{b}h{h}")
                ph.append(ps)
                # Pre-zero the last column: the final matmul only writes
                # lanes 0..MLAST-1 there; lanes MLAST..127 must read as 0.
                nc.vector.memset(ps[:, NCH - 1 : NCH], 0.0)
                for t in range(NT):
                    v0 = t * C
                    ct = min(C, V - v0)
                    tl = io.tile([128, C], F32, tag="io")
                    src = bass.AP(
                        x, b * S2 * V + h * 128 * V + v0, [[V, 128], [1, ct]]
                    )
                    nc.sync.dma_start(tl[:, 0:ct], src)
                    # bf16 copy, alternating between the scalar and vector
                    # engines (both otherwise idle): fp32 stationary operands
                    # disable fast-weight-load and choke the PE at
                    # ~430ns/chunk; bf16 weights stream 2x/cycle. PSUM still
                    # accumulates fp32, so only the input rounding (2^-9
                    # relative) is lost -- far inside the error budget.
                    tb = iob.tile([128, C], BF16, tag="iob")
                    if t % 3 == 0:
                        nc.scalar.activation(tb[:, 0:ct], tl[:, 0:ct], AF.Copy)
                    else:
                        nc.vector.tensor_copy(tb[:, 0:ct], tl[:, 0:ct])
                    for c in range(0, ct, 128):
                        m = min(128, ct - c)
                        col = (v0 + c) // 128
                        nc.tensor.matmul(
                            ps[0:m, col : col + 1],
                            tb[:, c : c + m],
                            ones[:, 0:1],
                            start=True,
                            stop=True,
                        )
            # Epilogue: fold the two seq halves, clean pads, reduce stats.
            acc = accp.tile([128, NCH], F32, tag=f"acc{b}")
            nc.vector.tensor_copy(acc[:], ph[0][:])
            nc.vector.tensor_add(acc[:], acc[:], ph[1][:])
            # Vocab 0: reference forces m[0] = 0.
            nc.vector.memset(acc[0:1, 0:1], 0.0)
            nc.vector.tensor_reduce(
                statsout[:, 4 * b : 4 * b + 1], acc[:], axis=AX.X, op=ALU.add
            )
            et = scr.tile([128, NCH], F32, tag="et")
            # exp(m) with the mean folded into the activation scale; accum_out
            # gives the per-partition row sum in the same instruction. The
            # NPAD zeros contribute exp(0)=1 each; host subtracts them.
            nc.scalar.activation(
                et[:],
                acc[:],
                AF.Exp,
                scale=1.0 / S2,
                accum_out=statsout[:, 4 * b + 1 : 4 * b + 2],
            )
            st2 = scr.tile([128, NCH], F32, tag="st2")
            nc.vector.tensor_mul(st2[:], wt[:], acc[:])
            nc.vector.tensor_reduce(
                statsout[:, 4 * b + 2 : 4 * b + 3], st2[:], axis=AX.X, op=ALU.add
            )
        nc.sync.dma_start(out[:], statsout[:])
    nc.compile()
    return nc


_NC = None


def _get_program():
    global _NC
    if _NC is None:
        _NC = build_program()
    return _NC


def keyword_counts_and_hot(keywords):
    """Per-row unique non-special keyword count + multi-hot in device layout.

    Device layout: vocab id v -> (partition v % 128, column v // 128).
    """
    kw = np.asarray(keywords)
    ks = np.zeros(B, np.int64)
    whot = np.zeros((B, 128, NCH), np.float32)
    for bb in range(B):
        row = kw[bb].astype(np.int64)
        row = np.where(np.isin(row, SPECIAL), 0, row)
        uniq = np.unique(row)
        uniq = uniq[uniq != 0]
        ks[bb] = len(uniq)
        whot[bb, uniq % 128, uniq // 128] = 1.0
    return ks, whot


def make_in_maps(logits, keywords):
    logits = np.ascontiguousarray(np.asarray(logits, dtype=np.float32))
    ks, whot = keyword_counts_and_hot(keywords)
    in_maps = []
    for c in range(NCORES):
        sl = slice(c * BLOC, (c + 1) * BLOC)
        in_maps.append(
            {
                "x": logits[sl].reshape(1, XLEN),
                "wh": whot[sl],
            }
        )
    return ks, in_maps


def loss_from_stats(results, ks):
    """Host finish: fold per-partition stats, apply closed form in float64."""
    ea = float(np.exp(ALPHA))
    total = 0.0
    for ci, r in enumerate(results):
        s = np.asarray(r["out"], np.float64)
        for bb in range(BLOC):
            g = ci * BLOC + bb
            a_raw = s[:, 4 * bb + 0].sum()
            e_raw = s[:, 4 * bb + 1].sum() - NPAD
            w_raw = s[:, 4 * bb + 2].sum()
            k = float(ks[g])
            d = (V - k) + k * ea
            lse = np.log(e_raw)
            sum_m = a_raw / S2
            sum_kw = w_raw / S2
            total += (
                lse
                - np.log(d)
                + ALPHA * k * ea / d
                - sum_m / d
                - (ea - 1.0) * sum_kw / d
            )
    return np.float32(total / B)


def kernel(logits, keywords):
    ks, in_maps = make_in_maps(logits, keywords)
    nc = _get_program()
    res = run_bass_kernel_spmd(nc, in_maps, list(range(NCORES)))
    return loss_from_stats(res.results, ks)
